# revision 31
# baseline (speedup 1.0000x reference)
"""Expert-parallel MoE SwiGLU kernel for 8 Trainium2 NeuronCores.

Strategy: expert parallelism with host-side dispatch/combine, plus
expert f-splitting for load balance. With S = _SPLIT slots per core,
each expert's [d, f] weight stacks are split into S f-slices placed on
S different cores; each core holds S slices (of S different experts)
-- the same 25.2MB fp16 weight footprint as one whole expert -- and
runs the dense SwiGLU pipeline per slice:
    yT_partial = w_down[fslice].T-blocks @ (silu(wg.T@xT) * (wu.T@xT))
The host sums the S partial outputs per expert. Slot capacities come
from the sorted expert loads (slot j serves the loads of rank j, S+j,
2S+j, ...), so the per-core PE work is Sum_j max-load(slot j) token
columns instead of S * max-load: for this routing that is 526 (S=2) /
1034 (S=4) vs 548 / 1096 -- a ~4% cut of the PE-bound steady state.

Matmul operands stream as fp16 (fp32 PSUM accumulation; ~6e-4 max
relative error vs the fp32 reference), halving the weight traffic.
fp8 (DoubleRow) was evaluated and rejected: every quantization site
alone (x, w_gate/up, t, w_down) exceeds the 2e-2 max-relative-error
budget (measured 2.8e-2..3.9e-2), and int8/uint8 matmul is not exposed
by the Bass API (float dtypes only).

Schedule design, from perfetto-trace supply modeling:
- DMA rings are blocked until the ~7.2us framework preamble ends, then
  HBM sustains ~0.30-0.42MB/us/core. Descriptor issue costs ~650ns of
  ring-engine time, and a matmul chain needs its group's whole weight
  set anyway, so weights stream as ONE descriptor per matrix per
  f-group, all on the sync ring -- the only engine queue with no
  compute ops that could block its in-order FIFO (the scalar queue
  stalls behind silu ACTIVATEs waiting on PSUM). Splitting the first
  wave across the gpsimd/scalar rings was tried and is a net loss:
  rings share the 16 DMA engines and the deep sync pipeline crowds the
  others out, so x arrives LATER.
- the first descriptors' completion semaphores lag their data by
  ~1.5-3us (early-ramp artifact; the 16 per-engine completion shares
  of a descriptor post late while the engines ramp). No ordering
  removes it, so the first real matmul lands at ~13.6us regardless;
  the warmup burst is sized to bridge the whole window at full clock.
- f-group widths are graduated (slot 0: 128x4, 256x2, 512x...; later
  slots: 512s): small groups in front so the first gate chain starts
  ~4us earlier than a uniform split allows, wide groups later to
  amortize issue cost.
- within a group, all gate chains+silus are emitted before up chains,
  matching the gate-before-up arrival order on the wire; the previous
  group's down-projection chains interleave into the up phase.
- each group's w_down descriptor is issued one group late (first use is
  one group later), pulling early gate/up arrivals forward.
- a ~38-matmul dummy burst at the start keeps the PE activity monitor
  fed so the clock is at 2.4GHz (not the cold 1.2GHz) when real work
  lands; without it the HAM revokes full clock for 7-13us. (Beware the
  P0 power-state downclock to ~2.0GHz -- it shows as ~147ns/matmul
  steady state instead of ~122ns and is run-to-run luck.)
- non-final slots' partial outputs stream out mid-kernel on the (idle)
  gpsimd ring; the final slot's adds write fp16 staging tiles streamed
  per d-tile on the sync/scalar rings, so the drain after the last
  matmul is ~1.5us.
- all SBUF working tiles are allocated at the max slot capacity and
  column-sliced per slot (constant shape per pool tag); tile pools use
  bufs>=3 for weights -- bufs=2 produced wrong results with
  consecutive small groups (tile lifetime spans 2 groups).

Steady state: matmuls issue every ~(W/2.4GHz + 8)ns, PE-bound at the
fp16 rate with zero mid-stream stalls.
"""

import numpy as np
from contextlib import ExitStack

D_MODEL = 1024
D_FF = 4096
N_EXPERTS = 8
N_CORES = 8

_ND = D_MODEL // 128  # 8 contraction chunks over d_model

import os as _os
_CDT = _os.environ.get("MOE_KERNEL_DTYPE", "float16")
_WARM = int(_os.environ.get("MOE_WARMUP", "38"))
# 4-way f-split measured best: S=2 leaves ~1.5us of load imbalance,
# S=8 loses ~12us to single-point x-panel insertions stalling the
# weight stream at every one-group slot transition.
_SPLIT = int(_os.environ.get("MOE_SPLIT", "4"))

_nc_cache = {}


def _np_cdt():
    if _CDT == "float16":
        return np.float16
    if _CDT == "bfloat16":
        import ml_dtypes
        return ml_dtypes.bfloat16
    return np.float32


def _groups_for(slot: int, f_local: int):
    """Graduated f-group widths per slot; must sum to f_local."""
    if slot == 0:
        if f_local <= 512:
            return [128] * (f_local // 128)
        g = [128, 128, 128, 128, 256, 256]
        rem = f_local - sum(g)
        assert rem >= 0 and rem % 512 == 0
        g += [512] * (rem // 512)
    else:
        assert f_local % 512 == 0
        g = [512] * (f_local // 512)
    return g


def _build_nc(Ws: tuple):
    """Build + schedule the per-core Bass program.

    Ws: per-slot token capacities (slot j serves one expert's f-slice
    of width D_FF/len(Ws) at up to Ws[j] tokens).
    """
    import concourse.bacc as bacc
    import concourse.tile as tile
    from concourse import mybir

    S = len(Ws)
    FL = D_FF // S           # f columns per slot
    NFL = FL // 128          # f-tiles per slot
    Wmax = max(Ws)
    Wtot = sum(Ws)
    xoff = [_ND * sum(Ws[:k]) for k in range(S)]

    f32 = mybir.dt.float32
    f32r = getattr(mybir.dt, _CDT)

    nc = bacc.Bacc("TRN2", target_bir_lowering=False, debug=False,
                   num_devices=N_CORES)
    # x/y: per-slot [128, ND*Ws[k]] panels concatenated along free dim
    xt = nc.dram_tensor("xt", [128, _ND * Wtot], f32r,
                        kind="ExternalInput").ap()
    # wg/wu: per-slot, per-group blocks [128, ND*Fg] concatenated
    wg = nc.dram_tensor("wg", [128, _ND * D_FF], f32r,
                        kind="ExternalInput").ap()
    wu = nc.dram_tensor("wu", [128, _ND * D_FF], f32r,
                        kind="ExternalInput").ap()
    # wd: f-tile-major [128, (S*NFL)*D]: line p holds rows (c*128+p)
    wd = nc.dram_tensor("wd", [128, S * NFL * D_MODEL], f32r,
                        kind="ExternalInput").ap()
    yt = nc.dram_tensor("yt", [128, _ND * Wtot], f32r,
                        kind="ExternalOutput").ap()

    with tile.TileContext(nc) as tc, ExitStack() as ctx:
        xpool = ctx.enter_context(tc.tile_pool(name="x", bufs=1))
        # weight pools are 3-deep = 3 groups of prefetch lead (bufs=2
        # produced wrong results when a tile's lifetime spans 2 groups;
        # bufs=4 for wgp was tried and is noise-neutral while costing
        # SBUF headroom)
        wgp = ctx.enter_context(tc.tile_pool(name="wgp", bufs=3))
        wup = ctx.enter_context(tc.tile_pool(name="wup", bufs=3))
        wdp = ctx.enter_context(tc.tile_pool(name="wdp", bufs=3))
        tp = ctx.enter_context(tc.tile_pool(name="tp", bufs=3))
        gap = ctx.enter_context(tc.tile_pool(name="gap", bufs=3))
        yp = ctx.enter_context(tc.tile_pool(name="yp", bufs=2))
        yap = ctx.enter_context(tc.tile_pool(name="yap", bufs=2))
        pg = ctx.enter_context(tc.tile_pool(name="pg", bufs=2, space="PSUM"))
        pu = ctx.enter_context(tc.tile_pool(name="pu", bufs=2, space="PSUM"))
        pd = ctx.enter_context(tc.tile_pool(name="pd", bufs=4, space="PSUM"))

        # Input panel tile; slot 0's panel streams first on the sync
        # ring (head of the priority order). Later slots' panels also
        # stream on sync, issued two groups before their slot starts:
        # right before the slot's weights they stall the weight stream
        # ~1.5us per transition, on the gpsimd ring from the start they
        # crowd out the critical first weight groups (rings share the
        # 16 DMA engines) -- two groups of prefetch lead absorbs the
        # ~1.6us insertion with neither stall.
        x_t = xpool.tile([128, _ND * Wtot], f32r, tag="x")
        # small dummy transfers on the otherwise-idle scalar/gpsimd
        # rings: concurrent multi-queue traffic makes the DMA engines
        # post the sync ring's completion shares promptly (measured
        # lag 1.4us with concurrent traffic vs 2.7us without)
        dmy1 = xpool.tile([128, 128], f32r, tag="dmy1", name="dmy1")
        dmy2 = xpool.tile([128, 128], f32r, tag="dmy2", name="dmy2")
        nc.scalar.dma_start(dmy1[:], wg[:, 0:128])
        nc.gpsimd.dma_start(dmy2[:], wu[:, 0:128])
        nc.sync.dma_start(x_t[:, :_ND * Ws[0]], xt[:, :_ND * Ws[0]])

        def x_sl(k, d):
            o = xoff[k] + d * Ws[k]
            return x_t[:, o:o + Ws[k]]

        scr_w = xpool.tile([128, 128], f32r, tag="scrw", name="scr_w")
        scr_x = xpool.tile([128, Wmax], f32r, tag="scrx", name="scr_x")
        nc.vector.memset(scr_w[:], 0.0)
        nc.vector.memset(scr_x[:], 0.0)
        scr_p = pd.tile([128, Wmax], f32, tag="pd", name="scr_p")
        scr_p2 = pd.tile([128, Wmax], f32, tag="pd", name="scr_p2")
        _scr = [scr_p, scr_p2]

        def emit_warmup(n):
            for i in range(n):
                nc.tensor.matmul(_scr[i % 2][:], scr_w[:], scr_x[:],
                                 start=True, stop=True)

        # opening burst: continuous PE activity bridging the gap between
        # ring unblock and the first weight group's arrival
        emit_warmup(_WARM)

        # flat group list across slots
        flat = []  # (slot, group-in-slot, fo_local, fg, n_groups_in_slot)
        for k in range(S):
            gl = _groups_for(k, FL)
            fo = 0
            for gi, fgw in enumerate(gl):
                flat.append((k, gi, fo, fgw, len(gl)))
                fo += fgw
        # Later slots' x panels are split into d-chunks issued between
        # group descriptors in the last few groups before their slot
        # starts: one big insertion anywhere stalls the weight stream
        # ~1.5-1.8us, ~0.4us chunks are absorbed by the prefetch slack.
        x_chunks = {}  # flat idx -> list of (slot, d_lo, d_hi)
        prev_end = 1
        for k in range(1, S):
            start = next(i for i, f in enumerate(flat) if f[0] == k)
            pts = list(range(max(prev_end, start - 4), start))
            prev_end = start
            dper = -(-_ND // len(pts))
            d0 = 0
            for p in pts:
                if d0 >= _ND:
                    break
                x_chunks.setdefault(p, []).append(
                    (k, d0, min(_ND, d0 + dper)))
                d0 += dper

        # y accumulators: tag per d-tile, rotated across slots
        def y_acc(dt):
            return yap.tile([128, Wmax], f32, tag=f"y{dt}",
                            name=f"y_acc{dt}")

        y_cur = [None] * _ND  # live accumulator tiles for current slot

        def emit_down(prev_meta, dts):
            (k, gi, t_tiles, wd_t, ngr) = prev_meta
            Wk = Ws[k]
            last = (gi == ngr - 1)
            nft = len(t_tiles)
            for dt in dts:
                pdt = pd.tile([128, Wmax], f32, tag="pd",
                              name=f"pd_{k}_{gi}_{dt}")
                for ft in range(nft):
                    nc.tensor.matmul(
                        pdt[:, :Wk],
                        wd_t[:, ft * D_MODEL + dt * 128:
                             ft * D_MODEL + dt * 128 + 128],
                        t_tiles[ft][:, :Wk],
                        start=(ft == 0), stop=(ft == nft - 1))
                if last:
                    # final add for this slot writes a compact fp16
                    # staging tile; non-final slots drain on the idle
                    # gpsimd ring mid-kernel, the final slot alternates
                    # sync/scalar for the end-of-kernel drain
                    y16 = yp.tile([128, Wmax], f32r, tag=f"o{dt}",
                                  name=f"y16_{k}_{dt}")
                    if gi == 0:
                        nc.vector.tensor_copy(y16[:, :Wk], pdt[:, :Wk])
                    else:
                        nc.vector.tensor_add(y16[:, :Wk],
                                             y_cur[dt][:, :Wk],
                                             pdt[:, :Wk])
                    if k == S - 1:
                        eng = nc.sync if dt % 2 == 0 else nc.scalar
                    else:
                        eng = nc.gpsimd
                    o = xoff[k] + dt * Wk
                    eng.dma_start(yt[:, o:o + Wk], y16[:, :Wk])
                elif gi == 0:
                    y_cur[dt] = y_acc(dt)
                    nc.vector.tensor_copy(y_cur[dt][:, :Wk], pdt[:, :Wk])
                else:
                    nc.vector.tensor_add(y_cur[dt][:, :Wk],
                                         y_cur[dt][:, :Wk], pdt[:, :Wk])

        prev = None     # meta of the previous f group
        prev_wd = None  # (wd dram col offset, ftg, tile) pending issue
        for fi, (k, gi, fo, fgw, ngr) in enumerate(flat):
            Wk = Ws[k]
            ftg = fgw // 128
            glob_fo = k * FL + fo          # global f offset
            wcol = _ND * glob_fo           # wg/wu dram column offset
            for (kx, dlo, dhi) in x_chunks.get(fi, ()):
                o0 = xoff[kx] + dlo * Ws[kx]
                o1 = xoff[kx] + dhi * Ws[kx]
                nc.sync.dma_start(x_t[:, o0:o1], xt[:, o0:o1])
            wg_t = wgp.tile([128, _ND * fgw], f32r, tag=f"wg{fgw}")
            wu_t = wup.tile([128, _ND * fgw], f32r, tag=f"wu{fgw}")
            wd_t = wdp.tile([128, ftg * D_MODEL], f32r, tag=f"wd{fgw}")
            nc.sync.dma_start(wg_t[:], wg[:, wcol:wcol + _ND * fgw])
            nc.sync.dma_start(wu_t[:], wu[:, wcol:wcol + _ND * fgw])
            if prev_wd is not None:
                # issue the PREVIOUS group's down weights now: they are
                # first consumed during THIS group's up phase, so delaying
                # them one group pulls every early gate/up arrival forward
                pcol, pftg, pwd_t = prev_wd
                nc.sync.dma_start(pwd_t[:], wd[:, pcol:pcol + pftg * D_MODEL])
            prev_wd = ((glob_fo // 128) * D_MODEL, ftg, wd_t)

            # all gate chains (+ silu) first: the group's first compute
            # depends only on the gate descriptor, which arrives first
            g_acts = []
            for ft in range(ftg):
                psg = pg.tile([128, Wmax], f32, tag="pg")
                for d in range(_ND):
                    nc.tensor.matmul(
                        psg[:, :Wk],
                        wg_t[:, d * fgw + ft * 128:d * fgw + ft * 128 + 128],
                        x_sl(k, d),
                        start=(d == 0), stop=(d == _ND - 1))
                g_act = gap.tile([128, Wmax], f32, tag=f"g{ft}")
                nc.scalar.activation(g_act[:, :Wk], psg[:, :Wk],
                                     mybir.ActivationFunctionType.Silu)
                g_acts.append(g_act)

            # up chains + swiglu muls, with the previous group's down
            # chains interleaved to spread PSUM/vector pressure
            t_tiles = []
            for ft in range(ftg):
                psu = pu.tile([128, Wmax], f32, tag="pu")
                for d in range(_ND):
                    nc.tensor.matmul(
                        psu[:, :Wk],
                        wu_t[:, d * fgw + ft * 128:d * fgw + ft * 128 + 128],
                        x_sl(k, d),
                        start=(d == 0), stop=(d == _ND - 1))
                t_t = tp.tile([128, Wmax], f32r, tag=f"t{ft}")
                nc.vector.tensor_mul(t_t[:, :Wk], g_acts[ft][:, :Wk],
                                     psu[:, :Wk])
                t_tiles.append(t_t)
                if prev is not None:
                    lo = _ND * ft // ftg
                    hi = _ND * (ft + 1) // ftg
                    emit_down(prev, range(lo, hi))
            prev = (k, gi, t_tiles, wd_t, ngr)
        pcol, pftg, pwd_t = prev_wd
        nc.sync.dma_start(pwd_t[:], wd[:, pcol:pcol + pftg * D_MODEL])
        emit_down(prev, range(_ND))

    nc.compile()
    return nc


def _pack_gu(w, groups):
    # w: [D, FL] f-slice -> [128, ND*FL] in per-group blocks:
    # block_g[p, d*Fg + j] = w[d*128+p, fo_g + j]
    FLw = w.shape[1]
    w = np.asarray(w).astype(_np_cdt()).reshape(_ND, 128, FLw)
    blocks = []
    fo = 0
    for fgw in groups:
        blk = w[:, :, fo:fo + fgw]          # [ND, 128, Fg]
        blocks.append(blk.transpose(1, 0, 2).reshape(128, _ND * fgw))
        fo += fgw
    return np.concatenate(blocks, axis=1)


def _pack_wd(w):
    # w: [FL, D] f-slice -> [128, NFL*D]: dram[p, c*D+dj] = w[c*128+p, dj]
    nfl = w.shape[0] // 128
    w = np.asarray(w).astype(_np_cdt())
    return w.reshape(nfl, 128, D_MODEL).transpose(1, 0, 2).reshape(
        128, nfl * D_MODEL)


def _run_spmd(nc, in_maps):
    from concourse.bass_utils import run_bass_kernel_spmd
    for attempt in range(3):
        try:
            return run_bass_kernel_spmd(nc, in_maps,
                                        core_ids=list(range(N_CORES)))
        except Exception:
            if attempt == 2:
                raise
            import time
            time.sleep(3.0)
            # best-effort recovery from a wedged device (NRT_TIMEOUT /
            # NRT_EXEC_UNIT_UNRECOVERABLE): ask the runtime to reset
            # cores on re-init and rebuild the jax backend
            _os.environ.setdefault("NEURON_RT_RESET_CORES", "1")
            try:
                import jax
                jax.clear_caches()
                jax.clear_backends()
            except Exception:
                pass


def _run_split(S, Ws, slot_exp, tok_lists, x_flat, w_gate, w_up, w_down,
               out_flat, accumulate):
    """Run the S-way f-split SPMD program.

    slot_exp: [n_cores][S] expert index per (core, slot). The S cores
    that share an expert hold its S f-slices in the same slot index.
    Ws: per-slot capacities. Partial outputs are summed into out_flat.
    """
    key = tuple(Ws)
    if key not in _nc_cache:
        _nc_cache[key] = _build_nc(key)
    nc = _nc_cache[key]

    FL = D_FF // S
    cdt = _np_cdt()
    D = x_flat.shape[1]
    Wtot = sum(Ws)
    xoffc = [sum(Ws[:k]) for k in range(S)]

    in_maps = []
    for c in range(N_CORES):
        sl = (c % S)  # which f-slice this core holds
        xt_c = np.zeros((128, _ND * Wtot), dtype=cdt)
        wg_blocks, wu_blocks, wd_blocks = [], [], []
        for k in range(S):
            e = slot_exp[c][k]
            toks = tok_lists[e]
            Wk = Ws[k]
            xe = np.zeros((D, Wk), dtype=cdt)
            xe[:, :len(toks)] = x_flat[toks].T.astype(cdt)
            xt_c[:, _ND * xoffc[k]:_ND * (xoffc[k] + Wk)] = \
                xe.reshape(_ND, 128, Wk).transpose(1, 0, 2).reshape(
                    128, _ND * Wk)
            fsl = slice(sl * FL, (sl + 1) * FL)
            groups = _groups_for(k, FL)
            wg_blocks.append(_pack_gu(w_gate[e][:, fsl], groups))
            wu_blocks.append(_pack_gu(w_up[e][:, fsl], groups))
            wd_blocks.append(_pack_wd(w_down[e][fsl, :]))
        in_maps.append({
            "xt": np.ascontiguousarray(xt_c),
            "wg": np.ascontiguousarray(np.concatenate(wg_blocks, axis=1)),
            "wu": np.ascontiguousarray(np.concatenate(wu_blocks, axis=1)),
            "wd": np.ascontiguousarray(np.concatenate(wd_blocks, axis=1)),
        })

    global _last_run
    _last_run = (nc, in_maps)
    res = _run_spmd(nc, in_maps)

    for c in range(N_CORES):
        y = res.results[c]["yt"].astype(np.float32)
        for k in range(S):
            e = slot_exp[c][k]
            toks = tok_lists[e]
            Wk = Ws[k]
            part = y[:, _ND * xoffc[k]:_ND * (xoffc[k] + Wk)].reshape(
                128, _ND, Wk).transpose(1, 0, 2).reshape(D, Wk)
            if accumulate:
                out_flat[toks] += part[:, :len(toks)].T
            else:
                out_flat[toks] = part[:, :len(toks)].T


def kernel(x, expert_idx, w_gate, w_up, w_down):
    x = np.asarray(x, dtype=np.float32)
    idx = np.asarray(expert_idx).astype(np.int64)
    B, S_, D = x.shape
    T = B * S_
    x_flat = np.ascontiguousarray(x.reshape(T, D))
    idx_flat = idx.reshape(T)

    tok_lists = [np.nonzero(idx_flat == e)[0] for e in range(N_EXPERTS)]
    loads = np.array([len(t) for t in tok_lists])
    cap = max(1, loads.max())
    out_flat = np.zeros((T, D), dtype=np.float32)

    if cap <= 448:
        S = _SPLIT
        ranks = np.argsort(-loads)  # experts by load, descending
        # slot k serves ranks [k*(8//S), (k+1)*(8//S)): capacity = the
        # largest load in the slot. Core c holds f-slice (c % S); the S
        # cores {g*S..g*S+S-1} of group g share the same S experts.
        ngrp = N_CORES // S
        Ws = tuple(max(16, int(loads[ranks[k * ngrp]])) for k in range(S))
        # round capacities up to even column counts (4-byte dma lines)
        Ws = tuple(w + (w & 1) for w in Ws)
        slot_exp = [[int(ranks[k * ngrp + (c // S)]) for k in range(S)]
                    for c in range(N_CORES)]
        _run_split(S, Ws, slot_exp, tok_lists, x_flat,
                   w_gate, w_up, w_down, out_flat, accumulate=(S > 1))
    else:
        # fallback for extreme routing imbalance: process tokens in
        # rounds of <=256 per expert with the unsplit program
        rounds = -(-cap // 256)
        for r in range(rounds):
            round_lists = [t[r * 256:(r + 1) * 256] for t in tok_lists]
            slot_exp = [[c] for c in range(N_CORES)]
            _run_split(1, (256,), slot_exp, round_lists, x_flat,
                       w_gate, w_up, w_down, out_flat, accumulate=False)

    return out_flat.reshape(B, S_, D)


# revision 32
# speedup vs baseline: 1.2061x; 1.2061x over previous
"""Expert-parallel MoE SwiGLU kernel for 8 Trainium2 NeuronCores.

Strategy: expert parallelism with host-side dispatch/combine, plus
expert f-splitting for load balance. With S = _SPLIT slots per core,
each expert's [d, f] weight stacks are split into S f-slices placed on
S different cores; each core holds S slices (of S different experts)
-- the same 25.2MB fp16 weight footprint as one whole expert -- and
runs the dense SwiGLU pipeline per slice:
    yT_partial = w_down[fslice].T-blocks @ (silu(wg.T@xT) * (wu.T@xT))
The host sums the S partial outputs per expert. Slot capacities come
from the sorted expert loads (slot j serves the loads of rank j, S+j,
2S+j, ...), so the per-core PE work is Sum_j max-load(slot j) token
columns instead of S * max-load: for this routing that is 526 (S=2) /
1034 (S=4) vs 548 / 1096 -- a ~4% cut of the PE-bound steady state.

Matmul operands stream as fp16 (fp32 PSUM accumulation; ~6e-4 max
relative error vs the fp32 reference), halving the weight traffic.
fp8 (DoubleRow) was evaluated and rejected: every quantization site
alone (x, w_gate/up, t, w_down) exceeds the 2e-2 max-relative-error
budget (measured 2.8e-2..3.9e-2), and int8/uint8 matmul is not exposed
by the Bass API (float dtypes only).

Schedule design, from perfetto-trace supply modeling:
- DMA rings are blocked until the ~7.2us framework preamble ends, then
  HBM sustains ~0.30-0.42MB/us/core. Descriptor issue costs ~650ns of
  ring-engine time, and a matmul chain needs its group's whole weight
  set anyway, so weights stream as ONE descriptor per matrix per
  f-group, all on the sync ring -- the only engine queue with no
  compute ops that could block its in-order FIFO (the scalar queue
  stalls behind silu ACTIVATEs waiting on PSUM). Splitting the first
  wave across the gpsimd/scalar rings was tried and is a net loss:
  rings share the 16 DMA engines and the deep sync pipeline crowds the
  others out, so x arrives LATER.
- the first descriptors' completion semaphores lag their data by
  ~1.5-3us (early-ramp artifact; the 16 per-engine completion shares
  of a descriptor post late while the engines ramp). No ordering
  removes it, so the first real matmul lands at ~13.6us regardless;
  the warmup burst is sized to bridge the whole window at full clock.
- f-group widths are graduated (slot 0: 128x4, 256x2, 512x...; later
  slots: 512s): small groups in front so the first gate chain starts
  ~4us earlier than a uniform split allows, wide groups later to
  amortize issue cost.
- within a group, all gate chains+silus are emitted before up chains,
  matching the gate-before-up arrival order on the wire; the previous
  group's down-projection chains interleave into the up phase.
- each group's w_down descriptor is issued one group late (first use is
  one group later), pulling early gate/up arrivals forward.
- a ~38-matmul dummy burst at the start keeps the PE activity monitor
  fed so the clock is at 2.4GHz (not the cold 1.2GHz) when real work
  lands; without it the HAM revokes full clock for 7-13us. (Beware the
  P0 power-state downclock to ~2.0GHz -- it shows as ~147ns/matmul
  steady state instead of ~122ns and is run-to-run luck.)
- non-final slots' partial outputs stream out mid-kernel on the (idle)
  gpsimd ring; the final slot's adds write fp16 staging tiles streamed
  per d-tile on the sync/scalar rings, so the drain after the last
  matmul is ~1.5us.
- all SBUF working tiles are allocated at the max slot capacity and
  column-sliced per slot (constant shape per pool tag); tile pools use
  bufs>=3 for weights -- bufs=2 produced wrong results with
  consecutive small groups (tile lifetime spans 2 groups).

Steady state: matmuls issue every ~(W/2.4GHz + 8)ns, PE-bound at the
fp16 rate with zero mid-stream stalls.
"""

import numpy as np
from contextlib import ExitStack

D_MODEL = 1024
D_FF = 4096
N_EXPERTS = 8
N_CORES = 8

_ND = D_MODEL // 128  # 8 contraction chunks over d_model

import os as _os
_CDT = _os.environ.get("MOE_KERNEL_DTYPE", "float16")
_WARM = int(_os.environ.get("MOE_WARMUP", "28"))
# 4-way f-split measured best: S=2 leaves ~1.5us of load imbalance,
# S=8 loses ~12us to single-point x-panel insertions stalling the
# weight stream at every one-group slot transition.
_SPLIT = int(_os.environ.get("MOE_SPLIT", "4"))

_nc_cache = {}


def _np_cdt():
    if _CDT == "float16":
        return np.float16
    if _CDT == "bfloat16":
        import ml_dtypes
        return ml_dtypes.bfloat16
    return np.float32


def _groups_for(slot: int, f_local: int):
    """Graduated f-group widths per slot; must sum to f_local."""
    if slot == 0:
        if f_local <= 512:
            return [128] * (f_local // 128)
        g = [128, 128, 128, 128, 256, 256]
        rem = f_local - sum(g)
        assert rem >= 0 and rem % 512 == 0
        g += [512] * (rem // 512)
    else:
        assert f_local % 512 == 0
        g = [512] * (f_local // 512)
    return g


def _build_nc(Ws: tuple):
    """Build + schedule the per-core Bass program.

    Ws: per-slot token capacities (slot j serves one expert's f-slice
    of width D_FF/len(Ws) at up to Ws[j] tokens).
    """
    import concourse.bacc as bacc
    import concourse.tile as tile
    from concourse import mybir

    S = len(Ws)
    FL = D_FF // S           # f columns per slot
    NFL = FL // 128          # f-tiles per slot
    Wmax = max(Ws)
    Wtot = sum(Ws)
    xoff = [_ND * sum(Ws[:k]) for k in range(S)]

    f32 = mybir.dt.float32
    f32r = getattr(mybir.dt, _CDT)

    nc = bacc.Bacc("TRN2", target_bir_lowering=False, debug=False,
                   num_devices=N_CORES)
    # x/y: per-slot [128, ND*Ws[k]] panels concatenated along free dim
    xt = nc.dram_tensor("xt", [128, _ND * Wtot], f32r,
                        kind="ExternalInput").ap()
    # wg/wu: per-slot, per-group blocks [128, ND*Fg] concatenated
    wg = nc.dram_tensor("wg", [128, _ND * D_FF], f32r,
                        kind="ExternalInput").ap()
    wu = nc.dram_tensor("wu", [128, _ND * D_FF], f32r,
                        kind="ExternalInput").ap()
    # wd: f-tile-major [128, (S*NFL)*D]: line p holds rows (c*128+p)
    wd = nc.dram_tensor("wd", [128, S * NFL * D_MODEL], f32r,
                        kind="ExternalInput").ap()
    yt = nc.dram_tensor("yt", [128, _ND * Wtot], f32r,
                        kind="ExternalOutput").ap()

    with tile.TileContext(nc) as tc, ExitStack() as ctx:
        xpool = ctx.enter_context(tc.tile_pool(name="x", bufs=1))
        # weight pools are 3-deep = 3 groups of prefetch lead (bufs=2
        # produced wrong results when a tile's lifetime spans 2 groups;
        # bufs=4 for wgp was tried and is noise-neutral while costing
        # SBUF headroom)
        wgp = ctx.enter_context(tc.tile_pool(name="wgp", bufs=3))
        wup = ctx.enter_context(tc.tile_pool(name="wup", bufs=3))
        wdp = ctx.enter_context(tc.tile_pool(name="wdp", bufs=3))
        tp = ctx.enter_context(tc.tile_pool(name="tp", bufs=3))
        gap = ctx.enter_context(tc.tile_pool(name="gap", bufs=3))
        yp = ctx.enter_context(tc.tile_pool(name="yp", bufs=2))
        yap = ctx.enter_context(tc.tile_pool(name="yap", bufs=2))
        pg = ctx.enter_context(tc.tile_pool(name="pg", bufs=2, space="PSUM"))
        pu = ctx.enter_context(tc.tile_pool(name="pu", bufs=2, space="PSUM"))
        pd = ctx.enter_context(tc.tile_pool(name="pd", bufs=4, space="PSUM"))

        # Input panel tile; slot 0's panel streams first on the sync
        # ring (head of the priority order). Later slots' panels also
        # stream on sync, issued two groups before their slot starts:
        # right before the slot's weights they stall the weight stream
        # ~1.5us per transition, on the gpsimd ring from the start they
        # crowd out the critical first weight groups (rings share the
        # 16 DMA engines) -- two groups of prefetch lead absorbs the
        # ~1.6us insertion with neither stall.
        x_t = xpool.tile([128, _ND * Wtot], f32r, tag="x")
        # small dummy transfers on the otherwise-idle scalar/gpsimd
        # rings: concurrent multi-queue traffic makes the DMA engines
        # post the sync ring's completion shares promptly (measured
        # lag 1.4us with concurrent traffic vs 2.7us without)
        dmy1 = xpool.tile([128, 128], f32r, tag="dmy1", name="dmy1")
        dmy2 = xpool.tile([128, 128], f32r, tag="dmy2", name="dmy2")
        nc.scalar.dma_start(dmy1[:], wg[:, 0:128])
        nc.gpsimd.dma_start(dmy2[:], wu[:, 0:128])
        nc.sync.dma_start(x_t[:, :_ND * Ws[0]], xt[:, :_ND * Ws[0]])

        def x_sl(k, d):
            o = xoff[k] + d * Ws[k]
            return x_t[:, o:o + Ws[k]]

        scr_w = xpool.tile([128, 128], f32r, tag="scrw", name="scr_w")
        scr_x = xpool.tile([128, Wmax], f32r, tag="scrx", name="scr_x")
        nc.vector.memset(scr_w[:], 0.0)
        nc.vector.memset(scr_x[:], 0.0)
        scr_p = pd.tile([128, Wmax], f32, tag="pd", name="scr_p")
        scr_p2 = pd.tile([128, Wmax], f32, tag="pd", name="scr_p2")
        _scr = [scr_p, scr_p2]

        def emit_warmup(n):
            for i in range(n):
                nc.tensor.matmul(_scr[i % 2][:], scr_w[:], scr_x[:],
                                 start=True, stop=True)

        # opening burst: continuous PE activity bridging the gap between
        # ring unblock and the first weight group's arrival
        emit_warmup(_WARM)

        # flat group list across slots
        flat = []  # (slot, group-in-slot, fo_local, fg, n_groups_in_slot)
        for k in range(S):
            gl = _groups_for(k, FL)
            fo = 0
            for gi, fgw in enumerate(gl):
                flat.append((k, gi, fo, fgw, len(gl)))
                fo += fgw
        # Later slots' x panels are split into d-chunks issued between
        # group descriptors in the last few groups before their slot
        # starts: one big insertion anywhere stalls the weight stream
        # ~1.5-1.8us, ~0.4us chunks are absorbed by the prefetch slack.
        x_chunks = {}  # flat idx -> list of (slot, d_lo, d_hi)
        prev_end = 1
        for k in range(1, S):
            start = next(i for i, f in enumerate(flat) if f[0] == k)
            pts = list(range(max(prev_end, start - 4), start))
            prev_end = start
            dper = -(-_ND // len(pts))
            d0 = 0
            for p in pts:
                if d0 >= _ND:
                    break
                x_chunks.setdefault(p, []).append(
                    (k, d0, min(_ND, d0 + dper)))
                d0 += dper

        # y accumulators: tag per d-tile, rotated across slots
        def y_acc(dt):
            return yap.tile([128, Wmax], f32, tag=f"y{dt}",
                            name=f"y_acc{dt}")

        y_cur = [None] * _ND  # live accumulator tiles for current slot

        def emit_down(prev_meta, dts):
            (k, gi, t_tiles, wd_t, ngr) = prev_meta
            Wk = Ws[k]
            last = (gi == ngr - 1)
            nft = len(t_tiles)
            for dt in dts:
                pdt = pd.tile([128, Wmax], f32, tag="pd",
                              name=f"pd_{k}_{gi}_{dt}")
                for ft in range(nft):
                    nc.tensor.matmul(
                        pdt[:, :Wk],
                        wd_t[:, ft * D_MODEL + dt * 128:
                             ft * D_MODEL + dt * 128 + 128],
                        t_tiles[ft][:, :Wk],
                        start=(ft == 0), stop=(ft == nft - 1))
                if last:
                    # final add for this slot writes a compact fp16
                    # staging tile; non-final slots drain on the idle
                    # gpsimd ring mid-kernel, the final slot alternates
                    # sync/scalar for the end-of-kernel drain
                    y16 = yp.tile([128, Wmax], f32r, tag=f"o{dt}",
                                  name=f"y16_{k}_{dt}")
                    if gi == 0:
                        nc.vector.tensor_copy(y16[:, :Wk], pdt[:, :Wk])
                    else:
                        nc.vector.tensor_add(y16[:, :Wk],
                                             y_cur[dt][:, :Wk],
                                             pdt[:, :Wk])
                    if k == S - 1:
                        eng = nc.sync if dt % 2 == 0 else nc.scalar
                    else:
                        eng = nc.gpsimd
                    o = xoff[k] + dt * Wk
                    eng.dma_start(yt[:, o:o + Wk], y16[:, :Wk])
                elif gi == 0:
                    y_cur[dt] = y_acc(dt)
                    nc.vector.tensor_copy(y_cur[dt][:, :Wk], pdt[:, :Wk])
                else:
                    nc.vector.tensor_add(y_cur[dt][:, :Wk],
                                         y_cur[dt][:, :Wk], pdt[:, :Wk])

        prev = None     # meta of the previous f group
        prev_wd = None  # (wd dram col offset, ftg, tile) pending issue
        for fi, (k, gi, fo, fgw, ngr) in enumerate(flat):
            Wk = Ws[k]
            ftg = fgw // 128
            glob_fo = k * FL + fo          # global f offset
            wcol = _ND * glob_fo           # wg/wu dram column offset
            for (kx, dlo, dhi) in x_chunks.get(fi, ()):
                o0 = xoff[kx] + dlo * Ws[kx]
                o1 = xoff[kx] + dhi * Ws[kx]
                nc.sync.dma_start(x_t[:, o0:o1], xt[:, o0:o1])
            wg_t = wgp.tile([128, _ND * fgw], f32r, tag=f"wg{fgw}")
            wu_t = wup.tile([128, _ND * fgw], f32r, tag=f"wu{fgw}")
            wd_t = wdp.tile([128, ftg * D_MODEL], f32r, tag=f"wd{fgw}")
            nc.sync.dma_start(wg_t[:], wg[:, wcol:wcol + _ND * fgw])
            nc.sync.dma_start(wu_t[:], wu[:, wcol:wcol + _ND * fgw])
            if prev_wd is not None:
                # issue the PREVIOUS group's down weights now: they are
                # first consumed during THIS group's up phase, so delaying
                # them one group pulls every early gate/up arrival forward
                pcol, pftg, pwd_t = prev_wd
                nc.sync.dma_start(pwd_t[:], wd[:, pcol:pcol + pftg * D_MODEL])
            prev_wd = ((glob_fo // 128) * D_MODEL, ftg, wd_t)

            # all gate chains (+ silu) first: the group's first compute
            # depends only on the gate descriptor, which arrives first
            g_acts = []
            for ft in range(ftg):
                psg = pg.tile([128, Wmax], f32, tag="pg")
                for d in range(_ND):
                    nc.tensor.matmul(
                        psg[:, :Wk],
                        wg_t[:, d * fgw + ft * 128:d * fgw + ft * 128 + 128],
                        x_sl(k, d),
                        start=(d == 0), stop=(d == _ND - 1))
                g_act = gap.tile([128, Wmax], f32, tag=f"g{ft}")
                nc.scalar.activation(g_act[:, :Wk], psg[:, :Wk],
                                     mybir.ActivationFunctionType.Silu)
                g_acts.append(g_act)

            # up chains + swiglu muls, with the previous group's down
            # chains interleaved to spread PSUM/vector pressure
            t_tiles = []
            for ft in range(ftg):
                psu = pu.tile([128, Wmax], f32, tag="pu")
                for d in range(_ND):
                    nc.tensor.matmul(
                        psu[:, :Wk],
                        wu_t[:, d * fgw + ft * 128:d * fgw + ft * 128 + 128],
                        x_sl(k, d),
                        start=(d == 0), stop=(d == _ND - 1))
                t_t = tp.tile([128, Wmax], f32r, tag=f"t{ft}")
                nc.vector.tensor_mul(t_t[:, :Wk], g_acts[ft][:, :Wk],
                                     psu[:, :Wk])
                t_tiles.append(t_t)
                if prev is not None:
                    lo = _ND * ft // ftg
                    hi = _ND * (ft + 1) // ftg
                    emit_down(prev, range(lo, hi))
            prev = (k, gi, t_tiles, wd_t, ngr)
        pcol, pftg, pwd_t = prev_wd
        nc.sync.dma_start(pwd_t[:], wd[:, pcol:pcol + pftg * D_MODEL])
        emit_down(prev, range(_ND))

    nc.compile()
    return nc


def _pack_gu(w, groups):
    # w: [D, FL] f-slice -> [128, ND*FL] in per-group blocks:
    # block_g[p, d*Fg + j] = w[d*128+p, fo_g + j]
    FLw = w.shape[1]
    w = np.asarray(w).astype(_np_cdt()).reshape(_ND, 128, FLw)
    blocks = []
    fo = 0
    for fgw in groups:
        blk = w[:, :, fo:fo + fgw]          # [ND, 128, Fg]
        blocks.append(blk.transpose(1, 0, 2).reshape(128, _ND * fgw))
        fo += fgw
    return np.concatenate(blocks, axis=1)


def _pack_wd(w):
    # w: [FL, D] f-slice -> [128, NFL*D]: dram[p, c*D+dj] = w[c*128+p, dj]
    nfl = w.shape[0] // 128
    w = np.asarray(w).astype(_np_cdt())
    return w.reshape(nfl, 128, D_MODEL).transpose(1, 0, 2).reshape(
        128, nfl * D_MODEL)


def _run_spmd(nc, in_maps):
    from concourse.bass_utils import run_bass_kernel_spmd
    for attempt in range(3):
        try:
            return run_bass_kernel_spmd(nc, in_maps,
                                        core_ids=list(range(N_CORES)))
        except Exception:
            if attempt == 2:
                raise
            import time
            time.sleep(3.0)
            # best-effort recovery from a wedged device (NRT_TIMEOUT /
            # NRT_EXEC_UNIT_UNRECOVERABLE): ask the runtime to reset
            # cores on re-init and rebuild the jax backend
            _os.environ.setdefault("NEURON_RT_RESET_CORES", "1")
            try:
                import jax
                jax.clear_caches()
                jax.clear_backends()
            except Exception:
                pass


def _run_split(S, Ws, slot_exp, tok_lists, x_flat, w_gate, w_up, w_down,
               out_flat, accumulate):
    """Run the S-way f-split SPMD program.

    slot_exp: [n_cores][S] expert index per (core, slot). The S cores
    that share an expert hold its S f-slices in the same slot index.
    Ws: per-slot capacities. Partial outputs are summed into out_flat.
    """
    key = tuple(Ws)
    if key not in _nc_cache:
        _nc_cache[key] = _build_nc(key)
    nc = _nc_cache[key]

    FL = D_FF // S
    cdt = _np_cdt()
    D = x_flat.shape[1]
    Wtot = sum(Ws)
    xoffc = [sum(Ws[:k]) for k in range(S)]

    in_maps = []
    for c in range(N_CORES):
        sl = (c % S)  # which f-slice this core holds
        xt_c = np.zeros((128, _ND * Wtot), dtype=cdt)
        wg_blocks, wu_blocks, wd_blocks = [], [], []
        for k in range(S):
            e = slot_exp[c][k]
            toks = tok_lists[e]
            Wk = Ws[k]
            xe = np.zeros((D, Wk), dtype=cdt)
            xe[:, :len(toks)] = x_flat[toks].T.astype(cdt)
            xt_c[:, _ND * xoffc[k]:_ND * (xoffc[k] + Wk)] = \
                xe.reshape(_ND, 128, Wk).transpose(1, 0, 2).reshape(
                    128, _ND * Wk)
            fsl = slice(sl * FL, (sl + 1) * FL)
            groups = _groups_for(k, FL)
            wg_blocks.append(_pack_gu(w_gate[e][:, fsl], groups))
            wu_blocks.append(_pack_gu(w_up[e][:, fsl], groups))
            wd_blocks.append(_pack_wd(w_down[e][fsl, :]))
        in_maps.append({
            "xt": np.ascontiguousarray(xt_c),
            "wg": np.ascontiguousarray(np.concatenate(wg_blocks, axis=1)),
            "wu": np.ascontiguousarray(np.concatenate(wu_blocks, axis=1)),
            "wd": np.ascontiguousarray(np.concatenate(wd_blocks, axis=1)),
        })

    global _last_run
    _last_run = (nc, in_maps)
    res = _run_spmd(nc, in_maps)

    for c in range(N_CORES):
        y = res.results[c]["yt"].astype(np.float32)
        for k in range(S):
            e = slot_exp[c][k]
            toks = tok_lists[e]
            Wk = Ws[k]
            part = y[:, _ND * xoffc[k]:_ND * (xoffc[k] + Wk)].reshape(
                128, _ND, Wk).transpose(1, 0, 2).reshape(D, Wk)
            if accumulate:
                out_flat[toks] += part[:, :len(toks)].T
            else:
                out_flat[toks] = part[:, :len(toks)].T


def kernel(x, expert_idx, w_gate, w_up, w_down):
    x = np.asarray(x, dtype=np.float32)
    idx = np.asarray(expert_idx).astype(np.int64)
    B, S_, D = x.shape
    T = B * S_
    x_flat = np.ascontiguousarray(x.reshape(T, D))
    idx_flat = idx.reshape(T)

    tok_lists = [np.nonzero(idx_flat == e)[0] for e in range(N_EXPERTS)]
    loads = np.array([len(t) for t in tok_lists])
    cap = max(1, loads.max())
    out_flat = np.zeros((T, D), dtype=np.float32)

    if cap <= 448:
        S = _SPLIT
        ranks = np.argsort(-loads)  # experts by load, descending
        # slot k serves ranks [k*(8//S), (k+1)*(8//S)): capacity = the
        # largest load in the slot. Core c holds f-slice (c % S); the S
        # cores {g*S..g*S+S-1} of group g share the same S experts.
        ngrp = N_CORES // S
        Ws = tuple(max(16, int(loads[ranks[k * ngrp]])) for k in range(S))
        # round capacities up to even column counts (4-byte dma lines)
        Ws = tuple(w + (w & 1) for w in Ws)
        slot_exp = [[int(ranks[k * ngrp + (c // S)]) for k in range(S)]
                    for c in range(N_CORES)]
        _run_split(S, Ws, slot_exp, tok_lists, x_flat,
                   w_gate, w_up, w_down, out_flat, accumulate=(S > 1))
    else:
        # fallback for extreme routing imbalance: process tokens in
        # rounds of <=256 per expert with the unsplit program
        rounds = -(-cap // 256)
        for r in range(rounds):
            round_lists = [t[r * 256:(r + 1) * 256] for t in tok_lists]
            slot_exp = [[c] for c in range(N_CORES)]
            _run_split(1, (256,), slot_exp, round_lists, x_flat,
                       w_gate, w_up, w_down, out_flat, accumulate=False)

    return out_flat.reshape(B, S_, D)


# revision 34
# speedup vs baseline: 1.2208x; 1.0122x over previous
"""Expert-parallel MoE SwiGLU kernel for 8 Trainium2 NeuronCores.

Strategy: expert parallelism with host-side dispatch/combine, plus
expert f-splitting for load balance. With S = _SPLIT slots per core,
each expert's [d, f] weight stacks are split into S f-slices placed on
S different cores; each core holds S slices (of S different experts)
-- the same 25.2MB fp16 weight footprint as one whole expert -- and
runs the dense SwiGLU pipeline per slice:
    yT_partial = w_down[fslice].T-blocks @ (silu(wg.T@xT) * (wu.T@xT))
The host sums the S partial outputs per expert. Slot capacities come
from the sorted expert loads (slot j serves the loads of rank j, S+j,
2S+j, ...), so the per-core PE work is Sum_j max-load(slot j) token
columns instead of S * max-load: for this routing that is 526 (S=2) /
1034 (S=4) vs 548 / 1096 -- a ~4% cut of the PE-bound steady state.

Matmul operands stream as fp16 (fp32 PSUM accumulation; ~6e-4 max
relative error vs the fp32 reference), halving the weight traffic.
fp8 (DoubleRow) was evaluated and rejected: every quantization site
alone (x, w_gate/up, t, w_down) exceeds the 2e-2 max-relative-error
budget (measured 2.8e-2..3.9e-2), and int8/uint8 matmul is not exposed
by the Bass API (float dtypes only).

Schedule design, from perfetto-trace supply modeling:
- DMA rings are blocked until the ~7.2us framework preamble ends, then
  HBM sustains ~0.30-0.42MB/us/core. Descriptor issue costs ~650ns of
  ring-engine time, and a matmul chain needs its group's whole weight
  set anyway, so weights stream as ONE descriptor per matrix per
  f-group, all on the sync ring -- the only engine queue with no
  compute ops that could block its in-order FIFO (the scalar queue
  stalls behind silu ACTIVATEs waiting on PSUM). Splitting the first
  wave across the gpsimd/scalar rings was tried and is a net loss:
  rings share the 16 DMA engines and the deep sync pipeline crowds the
  others out, so x arrives LATER.
- the first descriptors' completion semaphores lag their data by
  ~1.5-3us (early-ramp artifact; the 16 per-engine completion shares
  of a descriptor post late while the engines ramp). No ordering
  removes it, so the first real matmul lands at ~13.6us regardless;
  the warmup burst is sized to bridge the whole window at full clock.
- f-group widths are graduated (slot 0: 128x4, 256x2, 512x...; later
  slots: 512s): small groups in front so the first gate chain starts
  ~4us earlier than a uniform split allows, wide groups later to
  amortize issue cost.
- within a group, all gate chains+silus are emitted before up chains,
  matching the gate-before-up arrival order on the wire; the previous
  group's down-projection chains interleave into the up phase.
- each group's w_down descriptor is issued one group late (first use is
  one group later), pulling early gate/up arrivals forward.
- a ~38-matmul dummy burst at the start keeps the PE activity monitor
  fed so the clock is at 2.4GHz (not the cold 1.2GHz) when real work
  lands; without it the HAM revokes full clock for 7-13us. (Beware the
  P0 power-state downclock to ~2.0GHz -- it shows as ~147ns/matmul
  steady state instead of ~122ns and is run-to-run luck.)
- non-final slots' partial outputs stream out mid-kernel on the (idle)
  gpsimd ring; the final slot's adds write fp16 staging tiles streamed
  per d-tile on the sync/scalar rings, so the drain after the last
  matmul is ~1.5us.
- all SBUF working tiles are allocated at the max slot capacity and
  column-sliced per slot (constant shape per pool tag); tile pools use
  bufs>=3 for weights -- bufs=2 produced wrong results with
  consecutive small groups (tile lifetime spans 2 groups).

Steady state: matmuls issue every ~(W/2.4GHz + 8)ns, PE-bound at the
fp16 rate with zero mid-stream stalls.
"""

import numpy as np
from contextlib import ExitStack

D_MODEL = 1024
D_FF = 4096
N_EXPERTS = 8
N_CORES = 8

_ND = D_MODEL // 128  # 8 contraction chunks over d_model

import os as _os
_CDT = _os.environ.get("MOE_KERNEL_DTYPE", "float16")
_WARM = int(_os.environ.get("MOE_WARMUP", "38"))
# 4-way f-split measured best: S=2 leaves ~1.5us of load imbalance,
# S=8 loses ~12us to single-point x-panel insertions stalling the
# weight stream at every one-group slot transition.
_SPLIT = int(_os.environ.get("MOE_SPLIT", "4"))

_nc_cache = {}


def _np_cdt():
    if _CDT == "float16":
        return np.float16
    if _CDT == "bfloat16":
        import ml_dtypes
        return ml_dtypes.bfloat16
    return np.float32


def _groups_for(slot: int, f_local: int):
    """Graduated f-group widths per slot; must sum to f_local."""
    if slot == 0:
        if f_local <= 512:
            return [128] * (f_local // 128)
        g = [128, 128, 128, 128, 256, 256]
        rem = f_local - sum(g)
        assert rem >= 0 and rem % 512 == 0
        g += [512] * (rem // 512)
    else:
        assert f_local % 512 == 0
        g = [512] * (f_local // 512)
    return g


def _build_nc(Ws: tuple):
    """Build + schedule the per-core Bass program.

    Ws: per-slot token capacities (slot j serves one expert's f-slice
    of width D_FF/len(Ws) at up to Ws[j] tokens).
    """
    import concourse.bacc as bacc
    import concourse.tile as tile
    from concourse import mybir

    S = len(Ws)
    FL = D_FF // S           # f columns per slot
    NFL = FL // 128          # f-tiles per slot
    Wmax = max(Ws)
    Wtot = sum(Ws)
    xoff = [_ND * sum(Ws[:k]) for k in range(S)]

    f32 = mybir.dt.float32
    f32r = getattr(mybir.dt, _CDT)

    nc = bacc.Bacc("TRN2", target_bir_lowering=False, debug=False,
                   num_devices=N_CORES)
    # x/y: per-slot [128, ND*Ws[k]] panels concatenated along free dim
    xt = nc.dram_tensor("xt", [128, _ND * Wtot], f32r,
                        kind="ExternalInput").ap()
    # wg/wu: per-slot, per-group blocks [128, ND*Fg] concatenated
    wg = nc.dram_tensor("wg", [128, _ND * D_FF], f32r,
                        kind="ExternalInput").ap()
    wu = nc.dram_tensor("wu", [128, _ND * D_FF], f32r,
                        kind="ExternalInput").ap()
    # wd: f-tile-major [128, (S*NFL)*D]: line p holds rows (c*128+p)
    wd = nc.dram_tensor("wd", [128, S * NFL * D_MODEL], f32r,
                        kind="ExternalInput").ap()
    yt = nc.dram_tensor("yt", [128, _ND * Wtot], f32r,
                        kind="ExternalOutput").ap()

    with tile.TileContext(nc) as tc, ExitStack() as ctx:
        xpool = ctx.enter_context(tc.tile_pool(name="x", bufs=1))
        # weight pools are 3-deep = 3 groups of prefetch lead (bufs=2
        # produced wrong results when a tile's lifetime spans 2 groups;
        # bufs=4 for wgp was tried and is noise-neutral while costing
        # SBUF headroom)
        wgp = ctx.enter_context(tc.tile_pool(name="wgp", bufs=3))
        wup = ctx.enter_context(tc.tile_pool(name="wup", bufs=3))
        wdp = ctx.enter_context(tc.tile_pool(name="wdp", bufs=3))
        tp = ctx.enter_context(tc.tile_pool(name="tp", bufs=3))
        gap = ctx.enter_context(tc.tile_pool(name="gap", bufs=3))
        yp = ctx.enter_context(tc.tile_pool(name="yp", bufs=2))
        yap = ctx.enter_context(tc.tile_pool(name="yap", bufs=2))
        pg = ctx.enter_context(tc.tile_pool(name="pg", bufs=2, space="PSUM"))
        pu = ctx.enter_context(tc.tile_pool(name="pu", bufs=2, space="PSUM"))
        pd = ctx.enter_context(tc.tile_pool(name="pd", bufs=4, space="PSUM"))

        # Input panel tile; slot 0's panel streams first on the sync
        # ring (head of the priority order). Later slots' panels also
        # stream on sync, issued two groups before their slot starts:
        # right before the slot's weights they stall the weight stream
        # ~1.5us per transition, on the gpsimd ring from the start they
        # crowd out the critical first weight groups (rings share the
        # 16 DMA engines) -- two groups of prefetch lead absorbs the
        # ~1.6us insertion with neither stall.
        # (Dummy transfers on the scalar/gpsimd rings do make the sync
        # ring's completion semaphores post ~1.8us earlier, but the
        # early phase is SUPPLY-bound, not readiness-bound: starting
        # compute earlier just moves the stall into the group stream
        # while the dummies steal fill bandwidth -- measured net loss.)
        x_t = xpool.tile([128, _ND * Wtot], f32r, tag="x")
        nc.sync.dma_start(x_t[:, :_ND * Ws[0]], xt[:, :_ND * Ws[0]])

        def x_sl(k, d):
            o = xoff[k] + d * Ws[k]
            return x_t[:, o:o + Ws[k]]

        scr_w = xpool.tile([128, 128], f32r, tag="scrw", name="scr_w")
        scr_x = xpool.tile([128, Wmax], f32r, tag="scrx", name="scr_x")
        nc.vector.memset(scr_w[:], 0.0)
        nc.vector.memset(scr_x[:], 0.0)
        scr_p = pd.tile([128, Wmax], f32, tag="pd", name="scr_p")
        scr_p2 = pd.tile([128, Wmax], f32, tag="pd", name="scr_p2")
        _scr = [scr_p, scr_p2]

        def emit_warmup(n):
            for i in range(n):
                nc.tensor.matmul(_scr[i % 2][:], scr_w[:], scr_x[:],
                                 start=True, stop=True)

        # opening burst: continuous PE activity bridging the gap between
        # ring unblock and the first weight group's arrival
        emit_warmup(_WARM)

        # flat group list across slots
        flat = []  # (slot, group-in-slot, fo_local, fg, n_groups_in_slot)
        for k in range(S):
            gl = _groups_for(k, FL)
            fo = 0
            for gi, fgw in enumerate(gl):
                flat.append((k, gi, fo, fgw, len(gl)))
                fo += fgw
        # Later slots' x panels are split into d-chunks issued between
        # group descriptors in the last few groups before their slot
        # starts: one big insertion anywhere stalls the weight stream
        # ~1.5-1.8us, ~0.4us chunks are absorbed by the prefetch slack.
        x_chunks = {}  # flat idx -> list of (slot, d_lo, d_hi)
        prev_end = 1
        for k in range(1, S):
            start = next(i for i, f in enumerate(flat) if f[0] == k)
            pts = list(range(max(prev_end, start - 4), start))
            prev_end = start
            dper = -(-_ND // len(pts))
            d0 = 0
            for p in pts:
                if d0 >= _ND:
                    break
                x_chunks.setdefault(p, []).append(
                    (k, d0, min(_ND, d0 + dper)))
                d0 += dper

        # y accumulators: tag per d-tile, rotated across slots
        def y_acc(dt):
            return yap.tile([128, Wmax], f32, tag=f"y{dt}",
                            name=f"y_acc{dt}")

        y_cur = [None] * _ND  # live accumulator tiles for current slot

        def emit_down(prev_meta, dts):
            (k, gi, t_tiles, wd_t, ngr) = prev_meta
            Wk = Ws[k]
            last = (gi == ngr - 1)
            nft = len(t_tiles)
            for dt in dts:
                pdt = pd.tile([128, Wmax], f32, tag="pd",
                              name=f"pd_{k}_{gi}_{dt}")
                for ft in range(nft):
                    nc.tensor.matmul(
                        pdt[:, :Wk],
                        wd_t[:, ft * D_MODEL + dt * 128:
                             ft * D_MODEL + dt * 128 + 128],
                        t_tiles[ft][:, :Wk],
                        start=(ft == 0), stop=(ft == nft - 1))
                if last:
                    # final add for this slot writes a compact fp16
                    # staging tile; non-final slots drain on the idle
                    # gpsimd ring mid-kernel, the final slot alternates
                    # sync/scalar for the end-of-kernel drain
                    y16 = yp.tile([128, Wmax], f32r, tag=f"o{dt}",
                                  name=f"y16_{k}_{dt}")
                    if gi == 0:
                        nc.vector.tensor_copy(y16[:, :Wk], pdt[:, :Wk])
                    else:
                        nc.vector.tensor_add(y16[:, :Wk],
                                             y_cur[dt][:, :Wk],
                                             pdt[:, :Wk])
                    if k == S - 1:
                        eng = nc.sync if dt % 2 == 0 else nc.scalar
                    else:
                        eng = nc.gpsimd
                    o = xoff[k] + dt * Wk
                    eng.dma_start(yt[:, o:o + Wk], y16[:, :Wk])
                elif gi == 0:
                    y_cur[dt] = y_acc(dt)
                    nc.vector.tensor_copy(y_cur[dt][:, :Wk], pdt[:, :Wk])
                else:
                    nc.vector.tensor_add(y_cur[dt][:, :Wk],
                                         y_cur[dt][:, :Wk], pdt[:, :Wk])

        prev = None     # meta of the previous f group
        prev_wd = None  # (wd dram col offset, ftg, tile) pending issue
        for fi, (k, gi, fo, fgw, ngr) in enumerate(flat):
            Wk = Ws[k]
            ftg = fgw // 128
            glob_fo = k * FL + fo          # global f offset
            wcol = _ND * glob_fo           # wg/wu dram column offset
            for (kx, dlo, dhi) in x_chunks.get(fi, ()):
                o0 = xoff[kx] + dlo * Ws[kx]
                o1 = xoff[kx] + dhi * Ws[kx]
                nc.sync.dma_start(x_t[:, o0:o1], xt[:, o0:o1])
            wg_t = wgp.tile([128, _ND * fgw], f32r, tag=f"wg{fgw}")
            wu_t = wup.tile([128, _ND * fgw], f32r, tag=f"wu{fgw}")
            wd_t = wdp.tile([128, ftg * D_MODEL], f32r, tag=f"wd{fgw}")
            nc.sync.dma_start(wg_t[:], wg[:, wcol:wcol + _ND * fgw])
            nc.sync.dma_start(wu_t[:], wu[:, wcol:wcol + _ND * fgw])
            if prev_wd is not None:
                # issue the PREVIOUS group's down weights now: they are
                # first consumed during THIS group's up phase, so delaying
                # them one group pulls every early gate/up arrival forward
                pcol, pftg, pwd_t = prev_wd
                nc.sync.dma_start(pwd_t[:], wd[:, pcol:pcol + pftg * D_MODEL])
            prev_wd = ((glob_fo // 128) * D_MODEL, ftg, wd_t)

            # all gate chains (+ silu) first: the group's first compute
            # depends only on the gate descriptor, which arrives first
            g_acts = []
            for ft in range(ftg):
                psg = pg.tile([128, Wmax], f32, tag="pg")
                for d in range(_ND):
                    nc.tensor.matmul(
                        psg[:, :Wk],
                        wg_t[:, d * fgw + ft * 128:d * fgw + ft * 128 + 128],
                        x_sl(k, d),
                        start=(d == 0), stop=(d == _ND - 1))
                g_act = gap.tile([128, Wmax], f32, tag=f"g{ft}")
                nc.scalar.activation(g_act[:, :Wk], psg[:, :Wk],
                                     mybir.ActivationFunctionType.Silu)
                g_acts.append(g_act)

            # up chains + swiglu muls, with the previous group's down
            # chains interleaved to spread PSUM/vector pressure
            t_tiles = []
            for ft in range(ftg):
                psu = pu.tile([128, Wmax], f32, tag="pu")
                for d in range(_ND):
                    nc.tensor.matmul(
                        psu[:, :Wk],
                        wu_t[:, d * fgw + ft * 128:d * fgw + ft * 128 + 128],
                        x_sl(k, d),
                        start=(d == 0), stop=(d == _ND - 1))
                t_t = tp.tile([128, Wmax], f32r, tag=f"t{ft}")
                nc.vector.tensor_mul(t_t[:, :Wk], g_acts[ft][:, :Wk],
                                     psu[:, :Wk])
                t_tiles.append(t_t)
                if prev is not None:
                    lo = _ND * ft // ftg
                    hi = _ND * (ft + 1) // ftg
                    emit_down(prev, range(lo, hi))
            prev = (k, gi, t_tiles, wd_t, ngr)
        pcol, pftg, pwd_t = prev_wd
        nc.sync.dma_start(pwd_t[:], wd[:, pcol:pcol + pftg * D_MODEL])
        emit_down(prev, range(_ND))

    nc.compile()
    return nc


def _pack_gu(w, groups):
    # w: [D, FL] f-slice -> [128, ND*FL] in per-group blocks:
    # block_g[p, d*Fg + j] = w[d*128+p, fo_g + j]
    FLw = w.shape[1]
    w = np.asarray(w).astype(_np_cdt()).reshape(_ND, 128, FLw)
    blocks = []
    fo = 0
    for fgw in groups:
        blk = w[:, :, fo:fo + fgw]          # [ND, 128, Fg]
        blocks.append(blk.transpose(1, 0, 2).reshape(128, _ND * fgw))
        fo += fgw
    return np.concatenate(blocks, axis=1)


def _pack_wd(w):
    # w: [FL, D] f-slice -> [128, NFL*D]: dram[p, c*D+dj] = w[c*128+p, dj]
    nfl = w.shape[0] // 128
    w = np.asarray(w).astype(_np_cdt())
    return w.reshape(nfl, 128, D_MODEL).transpose(1, 0, 2).reshape(
        128, nfl * D_MODEL)


def _run_spmd(nc, in_maps):
    from concourse.bass_utils import run_bass_kernel_spmd
    for attempt in range(3):
        try:
            return run_bass_kernel_spmd(nc, in_maps,
                                        core_ids=list(range(N_CORES)))
        except Exception:
            if attempt == 2:
                raise
            import time
            time.sleep(3.0)
            # best-effort recovery from a wedged device (NRT_TIMEOUT /
            # NRT_EXEC_UNIT_UNRECOVERABLE): ask the runtime to reset
            # cores on re-init and rebuild the jax backend
            _os.environ.setdefault("NEURON_RT_RESET_CORES", "1")
            try:
                import jax
                jax.clear_caches()
                jax.clear_backends()
            except Exception:
                pass


def _run_split(S, Ws, slot_exp, tok_lists, x_flat, w_gate, w_up, w_down,
               out_flat, accumulate):
    """Run the S-way f-split SPMD program.

    slot_exp: [n_cores][S] expert index per (core, slot). The S cores
    that share an expert hold its S f-slices in the same slot index.
    Ws: per-slot capacities. Partial outputs are summed into out_flat.
    """
    key = tuple(Ws)
    if key not in _nc_cache:
        _nc_cache[key] = _build_nc(key)
    nc = _nc_cache[key]

    FL = D_FF // S
    cdt = _np_cdt()
    D = x_flat.shape[1]
    Wtot = sum(Ws)
    xoffc = [sum(Ws[:k]) for k in range(S)]

    in_maps = []
    for c in range(N_CORES):
        sl = (c % S)  # which f-slice this core holds
        xt_c = np.zeros((128, _ND * Wtot), dtype=cdt)
        wg_blocks, wu_blocks, wd_blocks = [], [], []
        for k in range(S):
            e = slot_exp[c][k]
            toks = tok_lists[e]
            Wk = Ws[k]
            xe = np.zeros((D, Wk), dtype=cdt)
            xe[:, :len(toks)] = x_flat[toks].T.astype(cdt)
            xt_c[:, _ND * xoffc[k]:_ND * (xoffc[k] + Wk)] = \
                xe.reshape(_ND, 128, Wk).transpose(1, 0, 2).reshape(
                    128, _ND * Wk)
            fsl = slice(sl * FL, (sl + 1) * FL)
            groups = _groups_for(k, FL)
            wg_blocks.append(_pack_gu(w_gate[e][:, fsl], groups))
            wu_blocks.append(_pack_gu(w_up[e][:, fsl], groups))
            wd_blocks.append(_pack_wd(w_down[e][fsl, :]))
        in_maps.append({
            "xt": np.ascontiguousarray(xt_c),
            "wg": np.ascontiguousarray(np.concatenate(wg_blocks, axis=1)),
            "wu": np.ascontiguousarray(np.concatenate(wu_blocks, axis=1)),
            "wd": np.ascontiguousarray(np.concatenate(wd_blocks, axis=1)),
        })

    global _last_run
    _last_run = (nc, in_maps)
    res = _run_spmd(nc, in_maps)

    for c in range(N_CORES):
        y = res.results[c]["yt"].astype(np.float32)
        for k in range(S):
            e = slot_exp[c][k]
            toks = tok_lists[e]
            Wk = Ws[k]
            part = y[:, _ND * xoffc[k]:_ND * (xoffc[k] + Wk)].reshape(
                128, _ND, Wk).transpose(1, 0, 2).reshape(D, Wk)
            if accumulate:
                out_flat[toks] += part[:, :len(toks)].T
            else:
                out_flat[toks] = part[:, :len(toks)].T


def kernel(x, expert_idx, w_gate, w_up, w_down):
    x = np.asarray(x, dtype=np.float32)
    idx = np.asarray(expert_idx).astype(np.int64)
    B, S_, D = x.shape
    T = B * S_
    x_flat = np.ascontiguousarray(x.reshape(T, D))
    idx_flat = idx.reshape(T)

    tok_lists = [np.nonzero(idx_flat == e)[0] for e in range(N_EXPERTS)]
    loads = np.array([len(t) for t in tok_lists])
    cap = max(1, loads.max())
    out_flat = np.zeros((T, D), dtype=np.float32)

    if cap <= 448:
        S = _SPLIT
        ranks = np.argsort(-loads)  # experts by load, descending
        # slot k serves ranks [k*(8//S), (k+1)*(8//S)): capacity = the
        # largest load in the slot. Core c holds f-slice (c % S); the S
        # cores {g*S..g*S+S-1} of group g share the same S experts.
        ngrp = N_CORES // S
        Ws = tuple(max(16, int(loads[ranks[k * ngrp]])) for k in range(S))
        # round capacities up to even column counts (4-byte dma lines)
        Ws = tuple(w + (w & 1) for w in Ws)
        slot_exp = [[int(ranks[k * ngrp + (c // S)]) for k in range(S)]
                    for c in range(N_CORES)]
        _run_split(S, Ws, slot_exp, tok_lists, x_flat,
                   w_gate, w_up, w_down, out_flat, accumulate=(S > 1))
    else:
        # fallback for extreme routing imbalance: process tokens in
        # rounds of <=256 per expert with the unsplit program
        rounds = -(-cap // 256)
        for r in range(rounds):
            round_lists = [t[r * 256:(r + 1) * 256] for t in tok_lists]
            slot_exp = [[c] for c in range(N_CORES)]
            _run_split(1, (256,), slot_exp, round_lists, x_flat,
                       w_gate, w_up, w_down, out_flat, accumulate=False)

    return out_flat.reshape(B, S_, D)
